# revision 1
# baseline (speedup 1.0000x reference)
"""Trainium2 Bass kernel for a 2-layer GRU (B=256, T=4096, I=26, H=128) + FC head.

Strategy (8 NeuronCores, data-parallel over batch, 32 rows per core):
  - All state kept transposed: [H=128 partitions, B=32 free].
  - Input-gate projections xg = W_ih @ x (+ all foldable biases) are computed in
    chunk-batched matmuls (64 timesteps at a time) and stored in SBUF (bf16).
  - The sequential recurrence runs 4096 rounds; layer 1 is software-pipelined
    two chunks behind layer 0, so the two layers form independent dependency
    chains that hide each other's latency.
  - Per round and layer: 3 W_hh matmuls + 1 identity-matmul (accumulates the
    precomputed xg into PSUM via has_written semantics), sigmoid on [r|z'] and
    tanh on n (ScalarE), the (hn+b_hn)*r product as one scalar_tensor_tensor
    (VectorE), and the h update split across VectorE/GpSimd.
  - z-gate weights/biases are pre-negated on the host so sigmoid yields
    z' = 1-z directly:  h' = h + z'*(n-h).
"""

import os
import sys
import functools

import numpy as np

sys.path.insert(0, "/opt/trn_rl_repo")

import ml_dtypes  # noqa: E402

BF16_NP = ml_dtypes.bfloat16

B, T, I, H, O = 256, 4096, 26, 128, 26
NCORES = 8
BL = B // NCORES  # 32 batch rows per core
P = 128
TC = 64  # timesteps per chunk
NCH = T // TC
LAG = 2  # layer-1 lag, in chunks
GCOLS = 192  # xga columns per round slot: [rz0(64) | xn0(32) | rz1(64) | xn1(32)]
NXGA = 3  # xga buffer rotation depth (must be > LAG)


def _build_nc(t_steps=T, tc=TC, lag=LAG):
    import concourse.bass as bass
    import concourse.mybir as mybir
    import concourse.tile as tile
    from concourse import bacc

    BF16 = mybir.dt.bfloat16
    F32 = mybir.dt.float32
    AF = mybir.ActivationFunctionType
    ALU = mybir.AluOpType

    nch = t_steps // tc
    nrounds = t_steps + lag * tc

    nc = bacc.Bacc(None)

    # ---- DRAM I/O ----
    xt = nc.dram_tensor("xt", [I + 1, t_steps, BL], BF16, kind="ExternalInput")
    h0t = nc.dram_tensor("h0t", [P, 2 * BL], BF16, kind="ExternalInput")
    w_hh0 = nc.dram_tensor("w_hh0", [P, 3 * H], BF16, kind="ExternalInput")
    w_hh1 = nc.dram_tensor("w_hh1", [P, 3 * H], BF16, kind="ExternalInput")
    w_ih0 = nc.dram_tensor("w_ih0", [I + 1, 3 * H], BF16, kind="ExternalInput")
    w_ih1a = nc.dram_tensor("w_ih1a", [P - 1, 3 * H], BF16, kind="ExternalInput")
    w_ih1b0 = nc.dram_tensor("w_ih1b0", [1, 3 * H], BF16, kind="ExternalInput")
    w_ih1b1 = nc.dram_tensor("w_ih1b1", [1, 3 * H], BF16, kind="ExternalInput")
    bhn = nc.dram_tensor("bhn", [P, 2], F32, kind="ExternalInput")
    fcw = nc.dram_tensor("fcw", [P, O], BF16, kind="ExternalInput")
    fcb = nc.dram_tensor("fcb", [O, 1], F32, kind="ExternalInput")
    ident = nc.dram_tensor("ident", [P, P], BF16, kind="ExternalInput")
    out = nc.dram_tensor("out", [O, BL], F32, kind="ExternalOutput")

    with tile.TileContext(nc) as tc_ctx:
        with (
            tc_ctx.tile_pool(name="singles", bufs=1) as singles,
            tc_ctx.tile_pool(name="xtp", bufs=2) as xtp,
            tc_ctx.tile_pool(name="h127p", bufs=2) as h127p,
            tc_ctx.tile_pool(name="stage", bufs=2, space="PSUM") as stage,
            tc_ctx.tile_pool(name="psA0", bufs=2, space="PSUM") as psA0,
            tc_ctx.tile_pool(name="psA1", bufs=2, space="PSUM") as psA1,
            tc_ctx.tile_pool(name="work", bufs=3) as work,
        ):
            # ---- constants to SBUF ----
            def load_const(dram, shape, dtype, tag):
                tl = singles.tile(shape, dtype, name=tag, tag=tag)
                nc.sync.dma_start(out=tl[:, :], in_=dram[:, :])
                return tl

            whh0s = load_const(w_hh0, [P, 3 * H], BF16, "whh0s")
            whh1s = load_const(w_hh1, [P, 3 * H], BF16, "whh1s")
            wih0s = load_const(w_ih0, [I + 1, 3 * H], BF16, "wih0s")
            wih1as = load_const(w_ih1a, [P - 1, 3 * H], BF16, "wih1as")
            wih1b0s = load_const(w_ih1b0, [1, 3 * H], BF16, "wih1b0s")
            wih1b1s = load_const(w_ih1b1, [1, 3 * H], BF16, "wih1b1s")
            bhns = load_const(bhn, [P, 2], F32, "bhns")
            fcws = load_const(fcw, [P, O], BF16, "fcws")
            fcbs = load_const(fcb, [O, 1], F32, "fcbs")
            idents = load_const(ident, [P, P], BF16, "idents")
            h_init = load_const(h0t, [P, 2 * BL], BF16, "h_init")

            ones_t = singles.tile([1, 512], BF16, name="ones_t", tag="ones_t")
            nc.vector.memset(ones_t[:, :], 1.0)

            # ---- persistent round buffers ----
            xga = [
                singles.tile(
                    [P, tc * GCOLS], BF16, name=f"xga{i}", tag=f"xga{i}"
                )
                for i in range(NXGA)
            ]
            hb0 = [
                singles.tile([P, tc * BL], BF16, name=f"hb0_{i}", tag=f"hb0_{i}")
                for i in range(2)
            ]
            hb1 = [
                singles.tile([P, tc * BL], BF16, name=f"hb1_{i}", tag=f"hb1_{i}")
                for i in range(2)
            ]

            NSL = 512 // BL  # rounds covered per 512-col staging tile (=16)

            def xg_copy(ps, buf, tt0, coloff, engine):
                # staging psum [128, 512] (= NSL rounds x BL cols, t-major) ->
                # strided round slots of an xga buffer, converting to bf16.
                dst = buf.rearrange("p (t c) -> p t c", c=GCOLS)[
                    :, tt0 : tt0 + NSL, coloff : coloff + BL
                ]
                src = ps.rearrange("p (t b) -> p t b", b=BL)
                if engine is nc.scalar:
                    engine.copy(dst, src)
                else:
                    engine.tensor_copy(dst, src)

            def emit_xg0(c):
                # layer-0 input gates for chunk c (consumed at rounds c*tc..)
                xtt = xtp.tile([I + 1, tc * BL], BF16, name="xtt", tag="xtt")
                nc.sync.dma_start(
                    out=xtt.rearrange("p (t b) -> p t b", b=BL),
                    in_=xt[:, c * tc : (c + 1) * tc, :],
                )
                buf = xga[c % NXGA]
                for g in range(3):
                    coloff = (0, 32, 64)[g]
                    for s in range(tc * BL // 512):
                        ps = stage.tile([P, 512], F32, name="stg", tag="stg")
                        nc.tensor.matmul(
                            ps[:, :],
                            wih0s[:, g * H : (g + 1) * H],
                            xtt[:, s * 512 : (s + 1) * 512],
                            start=True,
                            stop=True,
                        )
                        xg_copy(ps, buf, s * NSL, coloff, nc.vector)

            def emit_xg1(c):
                # layer-1 input gates for steps of chunk c-1; consumed at
                # rounds (c+1)*tc .. -> slots of xga[(c+1) % NXGA], offset +96
                hsrc = hb0[(c - 1) % 2]
                h127 = h127p.tile([1, tc * BL], BF16, name="h127", tag="h127")
                nc.sync.dma_start(out=h127[:, :], in_=hsrc[P - 1 : P, :])
                buf = xga[(c + 1) % NXGA]
                for g in range(3):
                    coloff = 96 + (0, 32, 64)[g]
                    for s in range(tc * BL // 512):
                        ps = stage.tile([P, 512], F32, name="stg", tag="stg")
                        nc.tensor.matmul(
                            ps[:, :],
                            wih1as[:, g * H : (g + 1) * H],
                            hsrc[0 : P - 1, s * 512 : (s + 1) * 512],
                            start=True,
                            stop=False,
                        )
                        nc.tensor.matmul(
                            ps[:, :],
                            wih1b0s[:, g * H : (g + 1) * H],
                            h127[:, s * 512 : (s + 1) * 512],
                            start=False,
                            stop=False,
                        )
                        nc.tensor.matmul(
                            ps[:, :],
                            wih1b1s[:, g * H : (g + 1) * H],
                            ones_t[:, :],
                            start=False,
                            stop=True,
                        )
                        xg_copy(ps, buf, s * NSL, coloff, nc.scalar)

            def emit_round_layer(l, step, c, tt):
                # one GRU step for layer l at global round c*tc+tt
                whh = whh0s if l == 0 else whh1s
                hb = hb0 if l == 0 else hb1
                psA = psA0 if l == 0 else psA1
                xoff = 0 if l == 0 else 96
                cs = step // tc
                ts = step % tc
                cur = hb[cs % 2]
                if step == 0:
                    hprev = h_init[:, l * BL : (l + 1) * BL]
                elif ts == 0:
                    hprev = hb[(cs - 1) % 2][:, (tc - 1) * BL : tc * BL]
                else:
                    hprev = cur[:, (ts - 1) * BL : ts * BL]

                xslot = xga[c % NXGA].rearrange("p (t c) -> p t c", c=GCOLS)[:, tt, :]

                A = psA.tile([P, 96], F32, name=f"A{l}", tag=f"A{l}")
                nc.tensor.matmul(
                    A[:, 0:32], whh[:, 0:128], hprev, start=True, stop=False
                )
                nc.tensor.matmul(
                    A[:, 32:64], whh[:, 128:256], hprev, start=False, stop=False
                )
                nc.tensor.matmul(
                    A[:, 64:96], whh[:, 256:384], hprev, start=False, stop=False
                )
                # accumulate xg(r|z) onto h-gates (identity matmul; has_written
                # is set for [0:64], so this adds; bias already folded into xg)
                nc.tensor.matmul(
                    A[:, 0:64],
                    idents[:, :],
                    xslot[:, xoff : xoff + 64],
                    start=False,
                    stop=True,
                )

                s_t = work.tile([P, 64], BF16, name=f"s{l}", tag=f"s{l}")
                nc.scalar.activation(s_t[:, :], A[:, 0:64], AF.Sigmoid)
                p_t = work.tile([P, BL], BF16, name=f"p{l}", tag=f"p{l}")
                nc.vector.scalar_tensor_tensor(
                    p_t[:, :],
                    A[:, 64:96],
                    bhns[:, l : l + 1],
                    s_t[:, 0:32],
                    ALU.add,
                    ALU.mult,
                )
                q_t = work.tile([P, BL], BF16, name=f"q{l}", tag=f"q{l}")
                nc.gpsimd.tensor_add(
                    q_t[:, :], p_t[:, :], xslot[:, xoff + 64 : xoff + 96]
                )
                n_t = work.tile([P, BL], BF16, name=f"n{l}", tag=f"n{l}")
                nc.scalar.activation(n_t[:, :], q_t[:, :], AF.Tanh)
                d_t = work.tile([P, BL], BF16, name=f"d{l}", tag=f"d{l}")
                nc.gpsimd.tensor_sub(d_t[:, :], n_t[:, :], hprev)
                f_t = work.tile([P, BL], BF16, name=f"f{l}", tag=f"f{l}")
                nc.vector.tensor_mul(f_t[:, :], d_t[:, :], s_t[:, 32:64])
                nc.vector.tensor_add(cur[:, ts * BL : (ts + 1) * BL], hprev, f_t[:, :])

            # ---- main static schedule ----
            for c in range(nch + lag):
                if c < nch:
                    emit_xg0(c)
                if 1 <= c and c - 1 < nch:
                    emit_xg1(c)
                for tt in range(tc):
                    r = c * tc + tt
                    if r < t_steps:
                        emit_round_layer(0, r, c, tt)
                    if r >= lag * tc:
                        emit_round_layer(1, r - lag * tc, c, tt)

            # ---- FC head on final h1 ----
            h_last = hb1[((t_steps - 1) // tc) % 2][:, (tc - 1) * BL : tc * BL]
            fps = stage.tile([O, BL], F32, name="fps", tag="fps", bufs=1)
            nc.tensor.matmul(fps[:, :], fcws[:, :], h_last, start=True, stop=True)
            fsb = singles.tile([O, BL], F32, name="fsb", tag="fsb")
            nc.scalar.activation(
                fsb[:, :], fps[:, :], AF.Identity, bias=fcbs[:, 0:1], scale=1.0
            )
            nc.sync.dma_start(out=out[:, :], in_=fsb[:, :])

    nc.compile()
    return nc


@functools.lru_cache(maxsize=2)
def _get_nc(t_steps=T):
    return _build_nc(t_steps=t_steps)


def _prep_shared(W_ih0, W_hh0, b_ih0, b_hh0, W_ih1, W_hh1, b_ih1, b_hh1, fc_w, fc_b):
    """Host-side weight packing (shared across cores)."""
    def gate_cat(wT, neg_z):
        # wT: [in, 3H] with gate blocks [r|z|n]; negate z block if asked
        w = wT.copy()
        if neg_z:
            w[:, H : 2 * H] = -w[:, H : 2 * H]
        return w

    whh0 = gate_cat(np.asarray(W_hh0).T.astype(np.float32), True)
    whh1 = gate_cat(np.asarray(W_hh1).T.astype(np.float32), True)

    wih0_base = gate_cat(np.asarray(W_ih0).T.astype(np.float32), True)  # [26, 384]
    brow0 = np.concatenate(
        [
            np.asarray(b_ih0[0:H]) + np.asarray(b_hh0[0:H]),
            -(np.asarray(b_ih0[H : 2 * H]) + np.asarray(b_hh0[H : 2 * H])),
            np.asarray(b_ih0[2 * H : 3 * H]),
        ]
    ).astype(np.float32)[None, :]
    wih0 = np.concatenate([wih0_base, brow0], axis=0)  # [27, 384]

    wih1_full = gate_cat(np.asarray(W_ih1).T.astype(np.float32), True)  # [128, 384]
    wih1a = wih1_full[0 : P - 1]
    wih1b0 = wih1_full[P - 1 : P]
    brow1 = np.concatenate(
        [
            np.asarray(b_ih1[0:H]) + np.asarray(b_hh1[0:H]),
            -(np.asarray(b_ih1[H : 2 * H]) + np.asarray(b_hh1[H : 2 * H])),
            np.asarray(b_ih1[2 * H : 3 * H]),
        ]
    ).astype(np.float32)[None, :]

    bhn_arr = np.stack(
        [np.asarray(b_hh0[2 * H : 3 * H]), np.asarray(b_hh1[2 * H : 3 * H])], axis=1
    ).astype(np.float32)

    shared = {
        "w_hh0": whh0.astype(BF16_NP),
        "w_hh1": whh1.astype(BF16_NP),
        "w_ih0": wih0.astype(BF16_NP),
        "w_ih1a": wih1a.astype(BF16_NP),
        "w_ih1b0": wih1b0.astype(BF16_NP),
        "w_ih1b1": brow1.astype(BF16_NP),
        "bhn": bhn_arr,
        "fcw": np.asarray(fc_w).T.astype(np.float32).astype(BF16_NP),  # [128, 26]
        "fcb": np.asarray(fc_b).astype(np.float32)[:, None],  # [26, 1]
        "ident": np.eye(P, dtype=np.float32).astype(BF16_NP),
    }
    return shared


def kernel(
    x,
    h0,
    W_ih0,
    W_hh0,
    b_ih0,
    b_hh0,
    W_ih1,
    W_hh1,
    b_ih1,
    b_hh1,
    fc_w,
    fc_b,
):
    from concourse.bass_utils import run_bass_kernel_spmd

    x = np.asarray(x, dtype=np.float32)
    h0 = np.asarray(h0, dtype=np.float32)
    t_steps = x.shape[1]

    shared = _prep_shared(
        W_ih0, W_hh0, b_ih0, b_hh0, W_ih1, W_hh1, b_ih1, b_hh1, fc_w, fc_b
    )

    in_maps = []
    for k in range(NCORES):
        bs = slice(k * BL, (k + 1) * BL)
        # xt: [27, T, 32]; xt[i,t,b] = x[b,t,i], row 26 = ones (bias row)
        xtk = np.empty((I + 1, t_steps, BL), dtype=np.float32)
        xtk[0:I] = x[bs].transpose(2, 1, 0)
        xtk[I] = 1.0
        h0tk = np.concatenate([h0[0, bs].T, h0[1, bs].T], axis=1)  # [128, 64]
        m = {"xt": xtk.astype(BF16_NP), "h0t": h0tk.astype(BF16_NP)}
        m.update(shared)
        in_maps.append(m)

    nc = _get_nc(t_steps)
    res = run_bass_kernel_spmd(nc, in_maps, core_ids=list(range(NCORES)))

    out_full = np.empty((B, O), dtype=np.float32)
    for k in range(NCORES):
        out_full[k * BL : (k + 1) * BL] = np.asarray(
            res.results[k]["out"], dtype=np.float32
        ).T
    return out_full

